# revision 18
# baseline (speedup 1.0000x reference)
"""Trainium2 Bass kernel for nn_Attention_68676527063657.

Full (unsharded) multi-head attention with a quirky causal mask:
  qw = q @ Wq.T; kw = k @ Wk.T; vw = v @ Wv.T   (per-head split, dk=dv=64)
  a  = (qw . kw)/8 - (1-v_mask)*1e10 - tril(ones)*1e10   (diag included!)
  o  = softmax(a) @ vw, then o *= q_mask

Sharding: core c in [0,8): batch b = c//4, head-group g = c%4 (heads 4g..4g+4).
Each core computes o[b, :, 256g:256g+256] independently; host gathers.

Device algorithm (per core), all matmuls in float32r (full-rate fp32-ish):
  - host pre-transposes q/k/v and weights so every contraction dim lands on
    SBUF partitions; no on-chip input transposes needed.
  - scores are computed TRANSPOSED: ST[k-chunk 128, q-block 512] so that
    softmax normalization needs no P transpose before the P@V matmul.
  - v_mask folds into the exp() activation bias (per-partition = per-k).
  - causal mask: k-chunks fully below the diagonal are skipped outright;
    diagonal chunks get a precomputed -8e10 triangular add in PSUM.
  - row-sums of P come free as an extra all-ones column of V (col 64).
  - rows with no visible key (always q=2047; plus any v_mask-degenerate
    rows) are fixed up with a per-row weighted average of vw computed
    on-device from host-provided weights.
"""

import numpy as np

B, L, D = 2, 2048, 1024
H, DK = 16, 64
HG = 4            # heads per core
E = HG * DK       # 256 per-core output features
NCORES = 8
J, QB = 4, 512    # q blocks
C, KB = 16, 128   # k chunks
BIG = 1e10

_CACHE = {}
PROFILE = False
LAST_EXEC_NS = None
LAST_TRACE = None
# "bf16" (fastest), "f32r" (most accurate), "hybrid" (f32r proj, bf16 attention)
PRECISION = "bf16"


def _build_program(nd, degen_qbs, precision):
    import concourse.bass as bass
    import concourse.mybir as mybir
    from concourse import bacc
    from concourse.tile import TileContext

    F32, F32R = mybir.dt.float32, mybir.dt.float32r
    BF16 = mybir.dt.bfloat16
    DT_PROJ = F32R if precision in ("f32r", "hybrid") else BF16
    DT_ATTN = F32R if precision == "f32r" else BF16
    TRT = BF16 if DT_ATTN == BF16 else F32
    AF = mybir.ActivationFunctionType
    ALU = mybir.AluOpType
    ts = bass.ts

    nc = bacc.Bacc(None)
    xq = nc.dram_tensor("xq", [D, L], DT_PROJ, kind="ExternalInput")
    xk = nc.dram_tensor("xk", [D, L], DT_PROJ, kind="ExternalInput")
    xv = nc.dram_tensor("xv", [D, L], DT_PROJ, kind="ExternalInput")
    wq = nc.dram_tensor("wq", [D, E], DT_PROJ, kind="ExternalInput")
    wk = nc.dram_tensor("wk", [D, E], DT_PROJ, kind="ExternalInput")
    wv = nc.dram_tensor("wv", [D, E], DT_PROJ, kind="ExternalInput")
    idn = nc.dram_tensor("idn", [128, 128], TRT, kind="ExternalInput")
    oz4 = nc.dram_tensor("oz4", [128, HG, 2], DT_ATTN, kind="ExternalInput")
    trm = nc.dram_tensor("trm", [128, 4, QB], F32, kind="ExternalInput")
    tr01 = nc.dram_tensor("tr01", [128, 4, QB], DT_ATTN, kind="ExternalInput")
    vmb = nc.dram_tensor("vmb", [128, C], F32, kind="ExternalInput")
    qmt = nc.dram_tensor("qmt", [128, C], F32, kind="ExternalInput")
    isdt = nc.dram_tensor("isdt", [128, C], F32, kind="ExternalInput")
    isdq = nc.dram_tensor("isdq", [128, C, nd], F32, kind="ExternalInput")
    wfx = nc.dram_tensor("wfx", [128, C, 2 * nd], DT_ATTN, kind="ExternalInput")
    o_d = nc.dram_tensor("o", [L, E], F32, kind="ExternalOutput")

    with TileContext(nc) as tc:
        with tc.tile_pool(name="consts", bufs=1) as consts, \
             tc.tile_pool(name="xt", bufs=9) as xtp, \
             tc.tile_pool(name="qk2", bufs=1) as qk2p, \
             tc.tile_pool(name="pp", bufs=8) as ppool, \
             tc.tile_pool(name="osb", bufs=3) as osbp, \
             tc.tile_pool(name="oall", bufs=17) as oallp, \
             tc.tile_pool(name="small", bufs=4) as small, \
             tc.tile_pool(name="fbp", bufs=1) as fbp:

            post_mask = (DT_ATTN != F32R)
            wdram = {"q": wq, "k": wk, "v": wv}
            wsb = {}

            # per-block activation tiles (fine-grained deps for interleaving)
            qw2 = [[qk2p.tile([128, QB], DT_ATTN, tag=f"qw2_{es}_{lc}",
                              name=f"qw2_{es}_{lc}") for lc in range(4)]
                   for es in range(2)]
            kw2 = [[qk2p.tile([128, QB], DT_ATTN, tag=f"kw2_{es}_{lc}",
                              name=f"kw2_{es}_{lc}") for lc in range(4)]
                   for es in range(2)]
            vw_c = [qk2p.tile([128, HG, 66], DT_ATTN, tag=f"vw_{c}",
                              name=f"vw_{c}") for c in range(C)]

            # ---------------- q/k projections (PE-dense prologue) ------------
            from contextlib import ExitStack
            _psa_scope = ExitStack()
            psA = _psa_scope.enter_context(
                tc.tile_pool(name="psA", bufs=8, space="PSUM"))
            xtv = None
            for nm in ("q", "k"):
                xin = {"q": xq, "k": xk}[nm]
                wsb[nm] = consts.tile([128, 8, E], DT_PROJ, tag=f"w_{nm}",
                                      name=f"wsb_{nm}")
                nc.sync.dma_start(
                    out=wsb[nm][:, :, :],
                    in_=wdram[nm].rearrange("(t p) e -> p t e", p=128))
                xt = [xtp.tile([128, L], DT_PROJ, tag="xt", name=f"xt_{nm}_{d2}")
                      for d2 in range(8)]
                if nm == "k":
                    # interleave the v input load so v-proj can start early
                    wsb["v"] = consts.tile([128, 8, E], DT_PROJ, tag="w_v",
                                           name="wsb_v")
                    nc.sync.dma_start(
                        out=wsb["v"][:, :, :],
                        in_=wdram["v"].rearrange("(t p) e -> p t e", p=128))
                    xtv = [xtp.tile([128, L], DT_PROJ, tag="xt",
                                    name=f"xt_v_{d2}") for d2 in range(8)]
                    for d in range(8):
                        nc.sync.dma_start(out=xt[d][:, :], in_=xin[ts(d, 128), :])
                        nc.sync.dma_start(out=xtv[d][:, :], in_=xv[ts(d, 128), :])
                else:
                    for d in range(8):
                        nc.sync.dma_start(out=xt[d][:, :], in_=xin[ts(d, 128), :])
                dst = qw2 if nm == "q" else kw2
                pss = [[psA.tile([128, QB], F32, tag="ppqk",
                                 name=f"ppqk_{nm}_{es}_{lc}")
                        for lc in range(4)] for es in range(2)]
                for d in range(8):
                    for es in range(2):
                        for lc in range(4):
                            nc.tensor.matmul(
                                pss[es][lc][:, :],
                                wsb[nm][:, d, ts(es, 128)],
                                xt[d][:, ts(lc, QB)],
                                start=(d == 0), stop=(d == 7))
                for es in range(2):
                    for lc in range(4):
                        nc.scalar.copy(out=dst[es][lc][:, :],
                                       in_=pss[es][lc][:, :])

            _psa_scope.close()
            _attn_scope = ExitStack()
            psST = _attn_scope.enter_context(
                tc.tile_pool(name="psST", bufs=2, space="PSUM"))
            psV = _attn_scope.enter_context(
                tc.tile_pool(name="psV", bufs=1, space="PSUM"))
            psOT = _attn_scope.enter_context(
                tc.tile_pool(name="psOT", bufs=2, space="PSUM"))
            psTR = _attn_scope.enter_context(
                tc.tile_pool(name="psTR", bufs=1, space="PSUM"))

            # late consts (needed from attention on)
            idt = consts.tile([128, 128], TRT, tag="idt")
            nc.sync.dma_start(out=idt[:, :], in_=idn[:, :])
            if post_mask:
                trim = consts.tile([128, 4, QB], DT_ATTN, tag="trim")
                nc.sync.dma_start(out=trim[:, :, :], in_=tr01[:, :, :])
            else:
                trim = consts.tile([128, 4, QB], F32, tag="trim")
                nc.sync.dma_start(out=trim[:, :, :], in_=trm[:, :, :])
            vmbt = consts.tile([128, C], F32, tag="vmbt")
            nc.sync.dma_start(out=vmbt[:, :], in_=vmb[:, :])
            qmtt = consts.tile([128, C], F32, tag="qmtt")
            nc.sync.dma_start(out=qmtt[:, :], in_=qmt[:, :])
            isdtt = consts.tile([128, C], F32, tag="isdtt")
            nc.sync.dma_start(out=isdtt[:, :], in_=isdt[:, :])
            isdqt = consts.tile([128, C, nd], F32, tag="isdqt")
            nc.sync.dma_start(out=isdqt[:, :, :], in_=isdq[:, :, :])
            wfxt = consts.tile([128, C, 2 * nd], DT_ATTN, tag="wfxt")
            nc.sync.dma_start(out=wfxt[:, :, :], in_=wfx[:, :, :])

            def emit_vproj(lt):
                ps = psV.tile([128, E], F32, tag="ppv", name=f"ppv_{lt}")
                for d in range(8):
                    nc.tensor.matmul(
                        ps[:, :], xtv[d][:, ts(lt, 128)], wsb["v"][:, d, :],
                        start=(d == 0), stop=(d == 7))
                for h in range(HG):
                    nc.vector.tensor_copy(out=vw_c[lt][:, h, 0:64],
                                          in_=ps[:, ts(h, 64)])
                nc.sync.dma_start(out=vw_c[lt][:, :, 64:66], in_=oz4[:, :, :])

            def emit_fix():
                fb = [[None] * nd for _ in range(HG)]
                for i in range(nd):
                    for h in range(HG):
                        pf = psOT.tile([2, 64], F32, tag="otp",
                                       name=f"pf_{i}_{h}")
                        for c in range(C):
                            nc.tensor.matmul(
                                pf[:, :],
                                wfxt[:, c, 2 * i:2 * i + 2],
                                vw_c[c][:, h, 0:64],
                                start=(c == 0), stop=(c == C - 1))
                        fr = small.tile([1, 64], F32, tag="fixrow")
                        nc.vector.tensor_copy(out=fr[:, :], in_=pf[0:1, :])
                        t = fbp.tile([128, 64], F32, tag=f"fb_{h}_{i}")
                        nc.gpsimd.partition_broadcast(t[:, :], fr[0:1, :])
                        fb[h][i] = t
                return fb

            # attention stream generator: yields after each chunk emission
            LAG = 5
            oall = [oallp.tile([128, E], F32, tag="oall", name=f"oall_{qb}")
                    for qb in range(C)]
            fb_holder = {}

            def attention_stream():
                for es in range(2):
                    for j in (range(J) if es == 0 else reversed(range(J))):
                        chunks = list(range(4 * j, C))
                        m = len(chunks)
                        ot2 = [psOT.tile([66, QB], F32, tag="otp",
                                         name=f"otp_{es}_{j}_{s2}")
                               for s2 in range(2)]
                        pbuf = [None] * m

                        def emit_ot(idx, c, ot2=ot2, pbuf=pbuf, m=m, j=j, es=es):
                            for sub in range(2):
                                nc.tensor.matmul(
                                    ot2[sub][:, :],
                                    vw_c[c][:, 2 * es + sub, :],
                                    pbuf[idx][:, ts(sub, QB)],
                                    start=(idx == 0), stop=(idx == m - 1))

                        for idx, c in enumerate(chunks):
                            st2 = psST.tile([128, 2 * QB], F32, tag="st",
                                            name=f"st_{es}_{j}_{c}")
                            for sub in range(2):
                                r0 = 64 * sub
                                nc.tensor.matmul(
                                    st2[:, ts(sub, QB)],
                                    kw2[es][c // 4][r0:r0 + 64, ts(c % 4, 128)],
                                    qw2[es][j][r0:r0 + 64, :],
                                    start=True, stop=True)
                            dd = c - 4 * j
                            diag = 0 <= dd < 4
                            if diag and not post_mask:
                                for sub in range(2):
                                    nc.vector.tensor_add(
                                        st2[:, ts(sub, QB)],
                                        st2[:, ts(sub, QB)], trim[:, dd, :])
                            p = ppool.tile([128, 2 * QB], DT_ATTN, tag="p")
                            nc.scalar.activation(
                                out=p[:, :], in_=st2[:, :], func=AF.Exp,
                                bias=vmbt[:, c:c + 1], scale=0.125)
                            if diag and post_mask:
                                off = 128 * dd
                                for sub in range(2):
                                    nc.vector.tensor_mul(
                                        p[:, sub * QB + off:(sub + 1) * QB],
                                        p[:, sub * QB + off:(sub + 1) * QB],
                                        trim[:, dd, off:QB])
                            pbuf[idx] = p
                            if idx >= LAG:
                                emit_ot(idx - LAG, chunks[idx - LAG])
                            yield
                        for idx in range(max(0, m - LAG), m):
                            emit_ot(idx, chunks[idx])

                        for sub in range(2):
                            h = 2 * es + sub
                            osb = osbp.tile([66, QB], TRT, tag="osb")
                            nc.vector.tensor_copy(out=osb[:, :],
                                                  in_=ot2[sub][:, :])
                            for t in range(4):
                                qb = 4 * j + t
                                tr = psTR.tile([128, 66], TRT, tag="tr")
                                nc.tensor.transpose(
                                    tr[:, :], osb[:, ts(t, 128)],
                                    idt[0:66, 0:66])
                                rc = small.tile([128, 1], F32, tag="rc")
                                if qb in degen_qbs:
                                    dn = small.tile([128, 1], F32, tag="dn")
                                    nc.vector.tensor_add(
                                        dn[:, :], tr[:, 64:65],
                                        isdtt[:, qb:qb + 1])
                                    nc.vector.reciprocal(rc[:, :], dn[:, :])
                                else:
                                    nc.vector.reciprocal(rc[:, :], tr[:, 64:65])
                                nc.vector.tensor_scalar(
                                    out=oall[qb][:, ts(h, 64)],
                                    in0=tr[:, 0:64], scalar1=rc[:, 0:1],
                                    scalar2=qmtt[:, qb:qb + 1],
                                    op0=ALU.mult, op1=ALU.mult)
                                for i in degen_qbs.get(qb, ()):
                                    fb = fb_holder["fb"]
                                    nc.vector.scalar_tensor_tensor(
                                        out=oall[qb][:, ts(h, 64)],
                                        in0=fb[h][i][:, :],
                                        scalar=isdqt[:, qb, i:i + 1],
                                        in1=oall[qb][:, ts(h, 64)],
                                        op0=ALU.mult, op1=ALU.add)
                                if h == HG - 1:
                                    nc.sync.dma_start(
                                        out=o_d[ts(qb, 128), :],
                                        in_=oall[qb][:, :])
                        yield

            # drive: interleave v-proj chunks with the attention stream 1:1
            stream = attention_stream()
            for lt in range(C):
                emit_vproj(lt)
                next(stream, None)
            fb_holder["fb"] = emit_fix()
            for _ in stream:
                pass
            _attn_scope.close()
    nc.finalize()
    return nc


def _host_prep(q, k, v, v_mask, q_mask, Wq, Wk, Wv, precision):
    """Per-core input maps + degenerate-row bookkeeping."""
    import ml_dtypes
    dt_proj = np.float32 if precision in ("f32r", "hybrid") else ml_dtypes.bfloat16
    dt_attn = np.float32 if precision == "f32r" else ml_dtypes.bfloat16
    f32 = np.float32
    q, k, v = (np.asarray(x, f32) for x in (q, k, v))
    v_mask, q_mask = np.asarray(v_mask, f32), np.asarray(q_mask, f32)
    Wq, Wk, Wv = (np.asarray(x, f32) for x in (Wq, Wk, Wv))

    idn = np.eye(128, dtype=f32)
    oz4 = np.zeros((128, HG, 2), f32)
    oz4[:, :, 0] = 1.0
    # trimask[p, dd, x] = -8e10 where x >= p + 128*dd  (k_global <= q_global)
    p_i = np.arange(128)[:, None, None]
    dd_i = np.arange(4)[None, :, None]
    x_i = np.arange(QB)[None, None, :]
    trm = np.where(x_i >= p_i + 128 * dd_i, f32(-8.0 * BIG), f32(0.0)).astype(f32)
    tr01 = np.where(x_i >= p_i + 128 * dd_i, f32(0.0), f32(1.0)).astype(f32)

    # degenerate rows per batch + fix weight vectors
    deg, wfix_cols, isd, isdq_cols = [], [], [], []
    for b in range(B):
        vm = v_mask[b]
        rows = [qq for qq in range(L)
                if qq == L - 1 or not vm[qq + 1:].any()]
        deg.append(rows)
        cols = []
        for qq in rows:
            single = np.zeros(L, f32)
            kk = np.arange(L)
            causal = kk <= qq
            pen = causal.astype(np.int64) + (vm == 0).astype(np.int64)
            m = pen == pen.min()   # max-attaining set under -BIG penalties
            single[m] = 1.0 / m.sum()
            cols.append(single)
        wfix_cols.append(cols)

    nd = max(len(r) for r in deg)
    degen_qbs = {}
    for b in range(B):
        for i, qq in enumerate(deg[b]):
            degen_qbs.setdefault(qq // 128, set()).add(i)
    degen_qbs = {qb: sorted(s) for qb, s in degen_qbs.items()}

    WqT, WkT, WvT = Wq.T.copy(), Wk.T.copy(), Wv.T.copy()
    in_maps = []
    for core in range(NCORES):
        b, g = divmod(core, HG)
        sl = slice(E * g, E * g + E)
        vm, qm = v_mask[b], q_mask[b]
        vmb = (-BIG * (1.0 - vm)).reshape(C, 128).T.astype(f32)
        qmt = qm.reshape(C, 128).T.astype(f32)
        isd_v = np.zeros(L, f32)
        isdq_v = np.zeros((L, nd), f32)
        wfx_v = np.zeros((L, 2 * nd), f32)
        for i, qq in enumerate(deg[b]):
            isd_v[qq] = 1.0
            isdq_v[qq, i] = qm[qq]
            wfx_v[:, 2 * i] = wfix_cols[b][i]
        in_maps.append({
            "xq": np.ascontiguousarray(q[b].T.astype(dt_proj)),
            "xk": np.ascontiguousarray(k[b].T.astype(dt_proj)),
            "xv": np.ascontiguousarray(v[b].T.astype(dt_proj)),
            "wq": np.ascontiguousarray(WqT[:, sl].astype(dt_proj)),
            "wk": np.ascontiguousarray(WkT[:, sl].astype(dt_proj)),
            "wv": np.ascontiguousarray(WvT[:, sl].astype(dt_proj)),
            "idn": idn.astype(dt_attn if precision != "f32r" else np.float32), "oz4": oz4.astype(dt_attn), "trm": trm,
            "tr01": tr01.astype(dt_attn),
            "vmb": vmb, "qmt": qmt,
            "isdt": isd_v.reshape(C, 128).T.copy(),
            "isdq": np.ascontiguousarray(
                isdq_v.reshape(C, 128, nd).transpose(1, 0, 2)),
            "wfx": np.ascontiguousarray(
                wfx_v.reshape(C, 128, 2 * nd).transpose(1, 0, 2).astype(dt_attn)),
        })
    return in_maps, nd, degen_qbs


def kernel(q, k, v, v_mask, q_mask, Wq, Wk, Wv):
    global LAST_EXEC_NS, LAST_TRACE
    from concourse.bass_utils import run_bass_kernel_spmd

    in_maps, nd, degen_qbs = _host_prep(q, k, v, v_mask, q_mask, Wq, Wk, Wv,
                                        PRECISION)
    key = (PRECISION, nd,
           tuple(sorted((qb, tuple(i)) for qb, i in degen_qbs.items())))
    if key not in _CACHE:
        _CACHE[key] = _build_program(nd, degen_qbs, PRECISION)
    nc = _CACHE[key]

    kwargs = {}
    if PROFILE:
        import sys, types
        sys.path.insert(0, "/root/.axon_site/trn_agent_boot")
        import trn_boot
        raw = trn_boot._ntff_profile_via_ctypes("/opt/axon/libaxon_pjrt.so")
        mod = types.ModuleType("antenv.axon_hooks")
        mod.get_axon_ntff_profile_hook = (
            lambda: (lambda out_dir, ids: raw(out_dir, None)))
        sys.modules["antenv.axon_hooks"] = mod
        kwargs = dict(trace=True)

    res = run_bass_kernel_spmd(nc, in_maps, core_ids=list(range(NCORES)), **kwargs)
    if PROFILE:
        LAST_EXEC_NS = res.exec_time_ns
        LAST_TRACE = (res.instructions_and_trace[1]
                      if res.instructions_and_trace else None)

    out = np.empty((B, L, H * DK), np.float32)
    for core in range(NCORES):
        b, g = divmod(core, HG)
        out[b, :, E * g:E * g + E] = res.results[core]["o"]
    return out


# revision 19
# speedup vs baseline: 1.0214x; 1.0214x over previous
"""Trainium2 Bass kernel for nn_Attention_68676527063657.

Full (unsharded) multi-head attention with a quirky causal mask:
  qw = q @ Wq.T; kw = k @ Wk.T; vw = v @ Wv.T   (per-head split, dk=dv=64)
  a  = (qw . kw)/8 - (1-v_mask)*1e10 - tril(ones)*1e10   (diag included!)
  o  = softmax(a) @ vw, then o *= q_mask

Sharding: core c in [0,8): batch b = c//4, head-group g = c%4 (heads 4g..4g+4).
Each core computes o[b, :, 256g:256g+256] independently; host gathers.

Device algorithm (per core), all matmuls in float32r (full-rate fp32-ish):
  - host pre-transposes q/k/v and weights so every contraction dim lands on
    SBUF partitions; no on-chip input transposes needed.
  - scores are computed TRANSPOSED: ST[k-chunk 128, q-block 512] so that
    softmax normalization needs no P transpose before the P@V matmul.
  - v_mask folds into the exp() activation bias (per-partition = per-k).
  - causal mask: k-chunks fully below the diagonal are skipped outright;
    diagonal chunks get a precomputed -8e10 triangular add in PSUM.
  - row-sums of P come free as an extra all-ones column of V (col 64).
  - rows with no visible key (always q=2047; plus any v_mask-degenerate
    rows) are fixed up with a per-row weighted average of vw computed
    on-device from host-provided weights.
"""

import numpy as np

B, L, D = 2, 2048, 1024
H, DK = 16, 64
HG = 4            # heads per core
E = HG * DK       # 256 per-core output features
NCORES = 8
J, QB = 4, 512    # q blocks
C, KB = 16, 128   # k chunks
BIG = 1e10

_CACHE = {}
PROFILE = False
LAST_EXEC_NS = None
LAST_TRACE = None
# "bf16" (fastest), "f32r" (most accurate), "hybrid" (f32r proj, bf16 attention)
PRECISION = "bf16"


def _build_program(nd, degen_qbs, precision):
    import concourse.bass as bass
    import concourse.mybir as mybir
    from concourse import bacc
    from concourse.tile import TileContext

    F32, F32R = mybir.dt.float32, mybir.dt.float32r
    BF16 = mybir.dt.bfloat16
    DT_PROJ = F32R if precision in ("f32r", "hybrid") else BF16
    DT_ATTN = F32R if precision == "f32r" else BF16
    TRT = BF16 if DT_ATTN == BF16 else F32
    AF = mybir.ActivationFunctionType
    ALU = mybir.AluOpType
    ts = bass.ts

    nc = bacc.Bacc(None)
    xq = nc.dram_tensor("xq", [D, L], DT_PROJ, kind="ExternalInput")
    xk = nc.dram_tensor("xk", [D, L], DT_PROJ, kind="ExternalInput")
    xv = nc.dram_tensor("xv", [D, L], DT_PROJ, kind="ExternalInput")
    wq = nc.dram_tensor("wq", [D, E], DT_PROJ, kind="ExternalInput")
    wk = nc.dram_tensor("wk", [D, E], DT_PROJ, kind="ExternalInput")
    wv = nc.dram_tensor("wv", [D, E], DT_PROJ, kind="ExternalInput")
    idn = nc.dram_tensor("idn", [128, 128], TRT, kind="ExternalInput")
    oz4 = nc.dram_tensor("oz4", [128, HG, 2], DT_ATTN, kind="ExternalInput")
    trm = nc.dram_tensor("trm", [128, 4, QB], F32, kind="ExternalInput")
    tr01 = nc.dram_tensor("tr01", [128, 4, QB], DT_ATTN, kind="ExternalInput")
    vmb = nc.dram_tensor("vmb", [128, C], F32, kind="ExternalInput")
    qmt = nc.dram_tensor("qmt", [128, C], F32, kind="ExternalInput")
    isdt = nc.dram_tensor("isdt", [128, C], F32, kind="ExternalInput")
    isdq = nc.dram_tensor("isdq", [128, C, nd], F32, kind="ExternalInput")
    wfx = nc.dram_tensor("wfx", [128, C, 2 * nd], DT_ATTN, kind="ExternalInput")
    o_d = nc.dram_tensor("o", [L, E], F32, kind="ExternalOutput")

    with TileContext(nc) as tc:
        with tc.tile_pool(name="consts", bufs=1) as consts, \
             tc.tile_pool(name="xt", bufs=17) as xtp, \
             tc.tile_pool(name="qk2", bufs=1) as qk2p, \
             tc.tile_pool(name="pp", bufs=8) as ppool, \
             tc.tile_pool(name="osb", bufs=3) as osbp, \
             tc.tile_pool(name="oall", bufs=17) as oallp, \
             tc.tile_pool(name="small", bufs=4) as small, \
             tc.tile_pool(name="fbp", bufs=1) as fbp:

            post_mask = (DT_ATTN != F32R)
            wdram = {"q": wq, "k": wk, "v": wv}
            wsb = {}

            # per-block activation tiles (fine-grained deps for interleaving)
            qw2 = [[qk2p.tile([128, QB], DT_ATTN, tag=f"qw2_{es}_{lc}",
                              name=f"qw2_{es}_{lc}") for lc in range(4)]
                   for es in range(2)]
            kw2 = [[qk2p.tile([128, QB], DT_ATTN, tag=f"kw2_{es}_{lc}",
                              name=f"kw2_{es}_{lc}") for lc in range(4)]
                   for es in range(2)]
            vw_c = [qk2p.tile([128, HG, 66], DT_ATTN, tag=f"vw_{c}",
                              name=f"vw_{c}") for c in range(C)]

            # ---------------- q/k projections (PE-dense prologue) ------------
            from contextlib import ExitStack
            _psa_scope = ExitStack()
            psA = _psa_scope.enter_context(
                tc.tile_pool(name="psA", bufs=8, space="PSUM"))
            xtv = None
            for nm in ("q", "k"):
                xin = {"q": xq, "k": xk}[nm]
                wsb[nm] = consts.tile([128, 8, E], DT_PROJ, tag=f"w_{nm}",
                                      name=f"wsb_{nm}")
                nc.sync.dma_start(
                    out=wsb[nm][:, :, :],
                    in_=wdram[nm].rearrange("(t p) e -> p t e", p=128))
                xt = [xtp.tile([128, L], DT_PROJ, tag="xt", name=f"xt_{nm}_{d2}")
                      for d2 in range(8)]
                if nm == "k":
                    # interleave the v input load so v-proj can start early
                    wsb["v"] = consts.tile([128, 8, E], DT_PROJ, tag="w_v",
                                           name="wsb_v")
                    nc.sync.dma_start(
                        out=wsb["v"][:, :, :],
                        in_=wdram["v"].rearrange("(t p) e -> p t e", p=128))
                    xtv = [xtp.tile([128, L], DT_PROJ, tag="xt",
                                    name=f"xt_v_{d2}") for d2 in range(8)]
                    for d in range(8):
                        nc.sync.dma_start(out=xt[d][:, :], in_=xin[ts(d, 128), :])
                        nc.sync.dma_start(out=xtv[d][:, :], in_=xv[ts(d, 128), :])
                else:
                    for d in range(8):
                        nc.sync.dma_start(out=xt[d][:, :], in_=xin[ts(d, 128), :])
                dst = qw2 if nm == "q" else kw2
                pss = [[psA.tile([128, QB], F32, tag="ppqk",
                                 name=f"ppqk_{nm}_{es}_{lc}")
                        for lc in range(4)] for es in range(2)]
                for d in range(8):
                    for es in range(2):
                        for lc in range(4):
                            nc.tensor.matmul(
                                pss[es][lc][:, :],
                                wsb[nm][:, d, ts(es, 128)],
                                xt[d][:, ts(lc, QB)],
                                start=(d == 0), stop=(d == 7))
                for es in range(2):
                    for lc in range(4):
                        nc.scalar.copy(out=dst[es][lc][:, :],
                                       in_=pss[es][lc][:, :])

            _psa_scope.close()
            _attn_scope = ExitStack()
            psST = _attn_scope.enter_context(
                tc.tile_pool(name="psST", bufs=2, space="PSUM"))
            psV = _attn_scope.enter_context(
                tc.tile_pool(name="psV", bufs=1, space="PSUM"))
            psOT = _attn_scope.enter_context(
                tc.tile_pool(name="psOT", bufs=2, space="PSUM"))
            psTR = _attn_scope.enter_context(
                tc.tile_pool(name="psTR", bufs=1, space="PSUM"))

            # late consts (needed from attention on)
            idt = consts.tile([128, 128], TRT, tag="idt")
            nc.sync.dma_start(out=idt[:, :], in_=idn[:, :])
            if post_mask:
                trim = consts.tile([128, 4, QB], DT_ATTN, tag="trim")
                nc.sync.dma_start(out=trim[:, :, :], in_=tr01[:, :, :])
            else:
                trim = consts.tile([128, 4, QB], F32, tag="trim")
                nc.sync.dma_start(out=trim[:, :, :], in_=trm[:, :, :])
            vmbt = consts.tile([128, C], F32, tag="vmbt")
            nc.sync.dma_start(out=vmbt[:, :], in_=vmb[:, :])
            qmtt = consts.tile([128, C], F32, tag="qmtt")
            nc.sync.dma_start(out=qmtt[:, :], in_=qmt[:, :])
            isdtt = consts.tile([128, C], F32, tag="isdtt")
            nc.sync.dma_start(out=isdtt[:, :], in_=isdt[:, :])
            isdqt = consts.tile([128, C, nd], F32, tag="isdqt")
            nc.sync.dma_start(out=isdqt[:, :, :], in_=isdq[:, :, :])
            wfxt = consts.tile([128, C, 2 * nd], DT_ATTN, tag="wfxt")
            nc.sync.dma_start(out=wfxt[:, :, :], in_=wfx[:, :, :])

            def emit_vproj(lt):
                ps = psV.tile([128, E], F32, tag="ppv", name=f"ppv_{lt}")
                for d in range(8):
                    nc.tensor.matmul(
                        ps[:, :], xtv[d][:, ts(lt, 128)], wsb["v"][:, d, :],
                        start=(d == 0), stop=(d == 7))
                for h in range(HG):
                    nc.vector.tensor_copy(out=vw_c[lt][:, h, 0:64],
                                          in_=ps[:, ts(h, 64)])
                nc.sync.dma_start(out=vw_c[lt][:, :, 64:66], in_=oz4[:, :, :])

            def emit_fix():
                fb = [[None] * nd for _ in range(HG)]
                for i in range(nd):
                    for h in range(HG):
                        pf = psOT.tile([2, 64], F32, tag="otp",
                                       name=f"pf_{i}_{h}")
                        for c in range(C):
                            nc.tensor.matmul(
                                pf[:, :],
                                wfxt[:, c, 2 * i:2 * i + 2],
                                vw_c[c][:, h, 0:64],
                                start=(c == 0), stop=(c == C - 1))
                        fr = small.tile([1, 64], F32, tag="fixrow")
                        nc.vector.tensor_copy(out=fr[:, :], in_=pf[0:1, :])
                        t = fbp.tile([128, 64], F32, tag=f"fb_{h}_{i}")
                        nc.gpsimd.partition_broadcast(t[:, :], fr[0:1, :])
                        fb[h][i] = t
                return fb

            # attention stream generator: yields after each chunk emission
            LAG = 5
            oall = [oallp.tile([128, E], F32, tag="oall", name=f"oall_{qb}")
                    for qb in range(C)]
            fb_holder = {}

            def attention_stream():
                for es in range(2):
                    for j in (range(J) if es == 0 else reversed(range(J))):
                        chunks = list(range(4 * j, C))
                        m = len(chunks)
                        ot2 = [psOT.tile([66, QB], F32, tag="otp",
                                         name=f"otp_{es}_{j}_{s2}")
                               for s2 in range(2)]
                        pbuf = [None] * m

                        def emit_ot(idx, c, ot2=ot2, pbuf=pbuf, m=m, j=j, es=es):
                            for sub in range(2):
                                nc.tensor.matmul(
                                    ot2[sub][:, :],
                                    vw_c[c][:, 2 * es + sub, :],
                                    pbuf[idx][:, ts(sub, QB)],
                                    start=(idx == 0), stop=(idx == m - 1))

                        for idx, c in enumerate(chunks):
                            st2 = psST.tile([128, 2 * QB], F32, tag="st",
                                            name=f"st_{es}_{j}_{c}")
                            for sub in range(2):
                                r0 = 64 * sub
                                nc.tensor.matmul(
                                    st2[:, ts(sub, QB)],
                                    kw2[es][c // 4][r0:r0 + 64, ts(c % 4, 128)],
                                    qw2[es][j][r0:r0 + 64, :],
                                    start=True, stop=True)
                            dd = c - 4 * j
                            diag = 0 <= dd < 4
                            if diag and not post_mask:
                                for sub in range(2):
                                    nc.vector.tensor_add(
                                        st2[:, ts(sub, QB)],
                                        st2[:, ts(sub, QB)], trim[:, dd, :])
                            p = ppool.tile([128, 2 * QB], DT_ATTN, tag="p")
                            nc.scalar.activation(
                                out=p[:, :], in_=st2[:, :], func=AF.Exp,
                                bias=vmbt[:, c:c + 1], scale=0.125)
                            if diag and post_mask:
                                off = 128 * dd
                                for sub in range(2):
                                    nc.vector.tensor_mul(
                                        p[:, sub * QB + off:(sub + 1) * QB],
                                        p[:, sub * QB + off:(sub + 1) * QB],
                                        trim[:, dd, off:QB])
                            pbuf[idx] = p
                            if idx >= LAG:
                                emit_ot(idx - LAG, chunks[idx - LAG])
                            yield
                        for idx in range(max(0, m - LAG), m):
                            emit_ot(idx, chunks[idx])

                        for sub in range(2):
                            h = 2 * es + sub
                            osb = osbp.tile([66, QB], TRT, tag="osb")
                            nc.vector.tensor_copy(out=osb[:, :],
                                                  in_=ot2[sub][:, :])
                            for t in range(4):
                                qb = 4 * j + t
                                tr = psTR.tile([128, 66], TRT, tag="tr")
                                nc.tensor.transpose(
                                    tr[:, :], osb[:, ts(t, 128)],
                                    idt[0:66, 0:66])
                                rc = small.tile([128, 1], F32, tag="rc")
                                if qb in degen_qbs:
                                    dn = small.tile([128, 1], F32, tag="dn")
                                    nc.vector.tensor_add(
                                        dn[:, :], tr[:, 64:65],
                                        isdtt[:, qb:qb + 1])
                                    nc.vector.reciprocal(rc[:, :], dn[:, :])
                                else:
                                    nc.vector.reciprocal(rc[:, :], tr[:, 64:65])
                                nc.vector.tensor_scalar(
                                    out=oall[qb][:, ts(h, 64)],
                                    in0=tr[:, 0:64], scalar1=rc[:, 0:1],
                                    scalar2=qmtt[:, qb:qb + 1],
                                    op0=ALU.mult, op1=ALU.mult)
                                for i in degen_qbs.get(qb, ()):
                                    fb = fb_holder["fb"]
                                    nc.vector.scalar_tensor_tensor(
                                        out=oall[qb][:, ts(h, 64)],
                                        in0=fb[h][i][:, :],
                                        scalar=isdqt[:, qb, i:i + 1],
                                        in1=oall[qb][:, ts(h, 64)],
                                        op0=ALU.mult, op1=ALU.add)
                                if h == HG - 1:
                                    nc.sync.dma_start(
                                        out=o_d[ts(qb, 128), :],
                                        in_=oall[qb][:, :])
                        yield

            # drive: interleave v-proj chunks with the attention stream 1:1
            stream = attention_stream()
            for lt in range(C):
                emit_vproj(lt)
                next(stream, None)
            fb_holder["fb"] = emit_fix()
            for _ in stream:
                pass
            _attn_scope.close()
    nc.finalize()
    return nc


def _host_prep(q, k, v, v_mask, q_mask, Wq, Wk, Wv, precision):
    """Per-core input maps + degenerate-row bookkeeping."""
    import ml_dtypes
    dt_proj = np.float32 if precision in ("f32r", "hybrid") else ml_dtypes.bfloat16
    dt_attn = np.float32 if precision == "f32r" else ml_dtypes.bfloat16
    f32 = np.float32
    q, k, v = (np.asarray(x, f32) for x in (q, k, v))
    v_mask, q_mask = np.asarray(v_mask, f32), np.asarray(q_mask, f32)
    Wq, Wk, Wv = (np.asarray(x, f32) for x in (Wq, Wk, Wv))

    idn = np.eye(128, dtype=f32)
    oz4 = np.zeros((128, HG, 2), f32)
    oz4[:, :, 0] = 1.0
    # trimask[p, dd, x] = -8e10 where x >= p + 128*dd  (k_global <= q_global)
    p_i = np.arange(128)[:, None, None]
    dd_i = np.arange(4)[None, :, None]
    x_i = np.arange(QB)[None, None, :]
    trm = np.where(x_i >= p_i + 128 * dd_i, f32(-8.0 * BIG), f32(0.0)).astype(f32)
    tr01 = np.where(x_i >= p_i + 128 * dd_i, f32(0.0), f32(1.0)).astype(f32)

    # degenerate rows per batch + fix weight vectors
    deg, wfix_cols, isd, isdq_cols = [], [], [], []
    for b in range(B):
        vm = v_mask[b]
        rows = [qq for qq in range(L)
                if qq == L - 1 or not vm[qq + 1:].any()]
        deg.append(rows)
        cols = []
        for qq in rows:
            single = np.zeros(L, f32)
            kk = np.arange(L)
            causal = kk <= qq
            pen = causal.astype(np.int64) + (vm == 0).astype(np.int64)
            m = pen == pen.min()   # max-attaining set under -BIG penalties
            single[m] = 1.0 / m.sum()
            cols.append(single)
        wfix_cols.append(cols)

    nd = max(len(r) for r in deg)
    degen_qbs = {}
    for b in range(B):
        for i, qq in enumerate(deg[b]):
            degen_qbs.setdefault(qq // 128, set()).add(i)
    degen_qbs = {qb: sorted(s) for qb, s in degen_qbs.items()}

    WqT, WkT, WvT = Wq.T.copy(), Wk.T.copy(), Wv.T.copy()
    in_maps = []
    for core in range(NCORES):
        b, g = divmod(core, HG)
        sl = slice(E * g, E * g + E)
        vm, qm = v_mask[b], q_mask[b]
        vmb = (-BIG * (1.0 - vm)).reshape(C, 128).T.astype(f32)
        qmt = qm.reshape(C, 128).T.astype(f32)
        isd_v = np.zeros(L, f32)
        isdq_v = np.zeros((L, nd), f32)
        wfx_v = np.zeros((L, 2 * nd), f32)
        for i, qq in enumerate(deg[b]):
            isd_v[qq] = 1.0
            isdq_v[qq, i] = qm[qq]
            wfx_v[:, 2 * i] = wfix_cols[b][i]
        in_maps.append({
            "xq": np.ascontiguousarray(q[b].T.astype(dt_proj)),
            "xk": np.ascontiguousarray(k[b].T.astype(dt_proj)),
            "xv": np.ascontiguousarray(v[b].T.astype(dt_proj)),
            "wq": np.ascontiguousarray(WqT[:, sl].astype(dt_proj)),
            "wk": np.ascontiguousarray(WkT[:, sl].astype(dt_proj)),
            "wv": np.ascontiguousarray(WvT[:, sl].astype(dt_proj)),
            "idn": idn.astype(dt_attn if precision != "f32r" else np.float32), "oz4": oz4.astype(dt_attn), "trm": trm,
            "tr01": tr01.astype(dt_attn),
            "vmb": vmb, "qmt": qmt,
            "isdt": isd_v.reshape(C, 128).T.copy(),
            "isdq": np.ascontiguousarray(
                isdq_v.reshape(C, 128, nd).transpose(1, 0, 2)),
            "wfx": np.ascontiguousarray(
                wfx_v.reshape(C, 128, 2 * nd).transpose(1, 0, 2).astype(dt_attn)),
        })
    return in_maps, nd, degen_qbs


def kernel(q, k, v, v_mask, q_mask, Wq, Wk, Wv):
    global LAST_EXEC_NS, LAST_TRACE
    from concourse.bass_utils import run_bass_kernel_spmd

    in_maps, nd, degen_qbs = _host_prep(q, k, v, v_mask, q_mask, Wq, Wk, Wv,
                                        PRECISION)
    key = (PRECISION, nd,
           tuple(sorted((qb, tuple(i)) for qb, i in degen_qbs.items())))
    if key not in _CACHE:
        _CACHE[key] = _build_program(nd, degen_qbs, PRECISION)
    nc = _CACHE[key]

    kwargs = {}
    if PROFILE:
        import sys, types
        sys.path.insert(0, "/root/.axon_site/trn_agent_boot")
        import trn_boot
        raw = trn_boot._ntff_profile_via_ctypes("/opt/axon/libaxon_pjrt.so")
        mod = types.ModuleType("antenv.axon_hooks")
        mod.get_axon_ntff_profile_hook = (
            lambda: (lambda out_dir, ids: raw(out_dir, None)))
        sys.modules["antenv.axon_hooks"] = mod
        kwargs = dict(trace=True)

    res = run_bass_kernel_spmd(nc, in_maps, core_ids=list(range(NCORES)), **kwargs)
    if PROFILE:
        LAST_EXEC_NS = res.exec_time_ns
        LAST_TRACE = (res.instructions_and_trace[1]
                      if res.instructions_and_trace else None)

    out = np.empty((B, L, H * DK), np.float32)
    for core in range(NCORES):
        b, g = divmod(core, HG)
        out[b, :, E * g:E * g + E] = res.results[core]["o"]
    return out


# revision 20
# speedup vs baseline: 1.0660x; 1.0436x over previous
"""Trainium2 Bass kernel for nn_Attention_68676527063657.

Full (unsharded) multi-head attention with a quirky causal mask:
  qw = q @ Wq.T; kw = k @ Wk.T; vw = v @ Wv.T   (per-head split, dk=dv=64)
  a  = (qw . kw)/8 - (1-v_mask)*1e10 - tril(ones)*1e10   (diag included!)
  o  = softmax(a) @ vw, then o *= q_mask

Sharding: core c in [0,8): batch b = c//4, head-group g = c%4 (heads 4g..4g+4).
Each core computes o[b, :, 256g:256g+256] independently; host gathers.

Device algorithm (per core), all matmuls in float32r (full-rate fp32-ish):
  - host pre-transposes q/k/v and weights so every contraction dim lands on
    SBUF partitions; no on-chip input transposes needed.
  - scores are computed TRANSPOSED: ST[k-chunk 128, q-block 512] so that
    softmax normalization needs no P transpose before the P@V matmul.
  - v_mask folds into the exp() activation bias (per-partition = per-k).
  - causal mask: k-chunks fully below the diagonal are skipped outright;
    diagonal chunks get a precomputed -8e10 triangular add in PSUM.
  - row-sums of P come free as an extra all-ones column of V (col 64).
  - rows with no visible key (always q=2047; plus any v_mask-degenerate
    rows) are fixed up with a per-row weighted average of vw computed
    on-device from host-provided weights.
"""

import numpy as np

B, L, D = 2, 2048, 1024
H, DK = 16, 64
HG = 4            # heads per core
E = HG * DK       # 256 per-core output features
NCORES = 8
J, QB = 4, 512    # q blocks
C, KB = 16, 128   # k chunks
BIG = 1e10

_CACHE = {}
PROFILE = False
LAST_EXEC_NS = None
LAST_TRACE = None
# "bf16" (fastest), "f32r" (most accurate), "hybrid" (f32r proj, bf16 attention)
PRECISION = "bf16"


def _build_program(nd, degen_qbs, precision):
    import concourse.bass as bass
    import concourse.mybir as mybir
    from concourse import bacc
    from concourse.tile import TileContext

    F32, F32R = mybir.dt.float32, mybir.dt.float32r
    BF16 = mybir.dt.bfloat16
    DT_PROJ = F32R if precision in ("f32r", "hybrid") else BF16
    DT_ATTN = F32R if precision == "f32r" else BF16
    TRT = BF16 if DT_ATTN == BF16 else F32
    AF = mybir.ActivationFunctionType
    ALU = mybir.AluOpType
    ts = bass.ts

    nc = bacc.Bacc(None)
    xq = nc.dram_tensor("xq", [D, L], DT_PROJ, kind="ExternalInput")
    xk = nc.dram_tensor("xk", [D, L], DT_PROJ, kind="ExternalInput")
    xv = nc.dram_tensor("xv", [D, L], DT_PROJ, kind="ExternalInput")
    wq = nc.dram_tensor("wq", [D, E], DT_PROJ, kind="ExternalInput")
    wk = nc.dram_tensor("wk", [D, E], DT_PROJ, kind="ExternalInput")
    wv = nc.dram_tensor("wv", [D, E], DT_PROJ, kind="ExternalInput")
    idn = nc.dram_tensor("idn", [128, 128], TRT, kind="ExternalInput")
    oz4 = nc.dram_tensor("oz4", [128, HG, 2], DT_ATTN, kind="ExternalInput")
    trm = nc.dram_tensor("trm", [128, 4, QB], F32, kind="ExternalInput")
    tr01 = nc.dram_tensor("tr01", [128, 4, QB], DT_ATTN, kind="ExternalInput")
    vmb = nc.dram_tensor("vmb", [128, C], F32, kind="ExternalInput")
    qmt = nc.dram_tensor("qmt", [128, C], F32, kind="ExternalInput")
    isdt = nc.dram_tensor("isdt", [128, C], F32, kind="ExternalInput")
    isdq = nc.dram_tensor("isdq", [128, C, nd], F32, kind="ExternalInput")
    wfx = nc.dram_tensor("wfx", [128, C, 2 * nd], DT_ATTN, kind="ExternalInput")
    o_d = nc.dram_tensor("o", [L, E], F32, kind="ExternalOutput")

    with TileContext(nc) as tc:
        with tc.tile_pool(name="consts", bufs=1) as consts, \
             tc.tile_pool(name="xt", bufs=17) as xtp, \
             tc.tile_pool(name="qk2", bufs=1) as qk2p, \
             tc.tile_pool(name="pp", bufs=8) as ppool, \
             tc.tile_pool(name="osb", bufs=3) as osbp, \
             tc.tile_pool(name="oall", bufs=17) as oallp, \
             tc.tile_pool(name="small", bufs=4) as small, \
             tc.tile_pool(name="fbp", bufs=1) as fbp:

            post_mask = (DT_ATTN != F32R)
            wdram = {"q": wq, "k": wk, "v": wv}
            wsb = {}

            # per-block activation tiles (fine-grained deps for interleaving)
            qw2 = [[qk2p.tile([128, QB], DT_ATTN, tag=f"qw2_{es}_{lc}",
                              name=f"qw2_{es}_{lc}") for lc in range(4)]
                   for es in range(2)]
            kw2 = [[qk2p.tile([128, QB], DT_ATTN, tag=f"kw2_{es}_{lc}",
                              name=f"kw2_{es}_{lc}") for lc in range(4)]
                   for es in range(2)]
            vw_c = [qk2p.tile([128, HG, 66], DT_ATTN, tag=f"vw_{c}",
                              name=f"vw_{c}") for c in range(C)]

            # ---------------- q/k projections (PE-dense prologue) ------------
            from contextlib import ExitStack
            _psa_scope = ExitStack()
            psA = _psa_scope.enter_context(
                tc.tile_pool(name="psA", bufs=8, space="PSUM"))
            xtv = None
            for nm in ("q", "k"):
                xin = {"q": xq, "k": xk}[nm]
                wsb[nm] = consts.tile([128, 8, E], DT_PROJ, tag=f"w_{nm}",
                                      name=f"wsb_{nm}")
                nc.sync.dma_start(
                    out=wsb[nm][:, :, :],
                    in_=wdram[nm].rearrange("(t p) e -> p t e", p=128))
                xt = [xtp.tile([128, L], DT_PROJ, tag="xt", name=f"xt_{nm}_{d2}")
                      for d2 in range(8)]
                for d in range(8):
                    nc.sync.dma_start(out=xt[d][:, :], in_=xin[ts(d, 128), :])
                if nm == "k":
                    wsb["v"] = consts.tile([128, 8, E], DT_PROJ, tag="w_v",
                                           name="wsb_v")
                    nc.sync.dma_start(
                        out=wsb["v"][:, :, :],
                        in_=wdram["v"].rearrange("(t p) e -> p t e", p=128))
                    xtv = [xtp.tile([128, L], DT_PROJ, tag="xt",
                                    name=f"xt_v_{d2}") for d2 in range(8)]
                    for d in range(8):
                        nc.sync.dma_start(out=xtv[d][:, :], in_=xv[ts(d, 128), :])
                dst = qw2 if nm == "q" else kw2
                pss = [[psA.tile([128, QB], F32, tag="ppqk",
                                 name=f"ppqk_{nm}_{es}_{lc}")
                        for lc in range(4)] for es in range(2)]
                for d in range(8):
                    for es in range(2):
                        for lc in range(4):
                            nc.tensor.matmul(
                                pss[es][lc][:, :],
                                wsb[nm][:, d, ts(es, 128)],
                                xt[d][:, ts(lc, QB)],
                                start=(d == 0), stop=(d == 7))
                for es in range(2):
                    for lc in range(4):
                        nc.scalar.copy(out=dst[es][lc][:, :],
                                       in_=pss[es][lc][:, :])

            _psa_scope.close()
            _attn_scope = ExitStack()
            psST = _attn_scope.enter_context(
                tc.tile_pool(name="psST", bufs=2, space="PSUM"))
            psV = _attn_scope.enter_context(
                tc.tile_pool(name="psV", bufs=1, space="PSUM"))
            psOT = _attn_scope.enter_context(
                tc.tile_pool(name="psOT", bufs=2, space="PSUM"))
            psTR = _attn_scope.enter_context(
                tc.tile_pool(name="psTR", bufs=1, space="PSUM"))

            # late consts (needed from attention on)
            idt = consts.tile([128, 128], TRT, tag="idt")
            nc.sync.dma_start(out=idt[:, :], in_=idn[:, :])
            if post_mask:
                trim = consts.tile([128, 4, QB], DT_ATTN, tag="trim")
                nc.sync.dma_start(out=trim[:, :, :], in_=tr01[:, :, :])
            else:
                trim = consts.tile([128, 4, QB], F32, tag="trim")
                nc.sync.dma_start(out=trim[:, :, :], in_=trm[:, :, :])
            vmbt = consts.tile([128, C], F32, tag="vmbt")
            nc.sync.dma_start(out=vmbt[:, :], in_=vmb[:, :])
            qmtt = consts.tile([128, C], F32, tag="qmtt")
            nc.sync.dma_start(out=qmtt[:, :], in_=qmt[:, :])
            isdtt = consts.tile([128, C], F32, tag="isdtt")
            nc.sync.dma_start(out=isdtt[:, :], in_=isdt[:, :])
            isdqt = consts.tile([128, C, nd], F32, tag="isdqt")
            nc.sync.dma_start(out=isdqt[:, :, :], in_=isdq[:, :, :])
            wfxt = consts.tile([128, C, 2 * nd], DT_ATTN, tag="wfxt")
            nc.sync.dma_start(out=wfxt[:, :, :], in_=wfx[:, :, :])

            def emit_vproj(lt):
                ps = psV.tile([128, E], F32, tag="ppv", name=f"ppv_{lt}")
                for d in range(8):
                    nc.tensor.matmul(
                        ps[:, :], xtv[d][:, ts(lt, 128)], wsb["v"][:, d, :],
                        start=(d == 0), stop=(d == 7))
                for h in range(HG):
                    nc.vector.tensor_copy(out=vw_c[lt][:, h, 0:64],
                                          in_=ps[:, ts(h, 64)])
                nc.sync.dma_start(out=vw_c[lt][:, :, 64:66], in_=oz4[:, :, :])

            def emit_fix():
                fb = [[None] * nd for _ in range(HG)]
                for i in range(nd):
                    for h in range(HG):
                        pf = psOT.tile([2, 64], F32, tag="otp",
                                       name=f"pf_{i}_{h}")
                        for c in range(C):
                            nc.tensor.matmul(
                                pf[:, :],
                                wfxt[:, c, 2 * i:2 * i + 2],
                                vw_c[c][:, h, 0:64],
                                start=(c == 0), stop=(c == C - 1))
                        fr = small.tile([1, 64], F32, tag="fixrow")
                        nc.vector.tensor_copy(out=fr[:, :], in_=pf[0:1, :])
                        t = fbp.tile([128, 64], F32, tag=f"fb_{h}_{i}")
                        nc.gpsimd.partition_broadcast(t[:, :], fr[0:1, :])
                        fb[h][i] = t
                return fb

            # attention stream generator: yields after each chunk emission
            LAG = 5
            oall = [oallp.tile([128, E], F32, tag="oall", name=f"oall_{qb}")
                    for qb in range(C)]
            fb_holder = {}

            def attention_stream():
                for es in range(2):
                    for j in (range(J) if es == 0 else reversed(range(J))):
                        chunks = list(range(4 * j, C))
                        m = len(chunks)
                        ot2 = [psOT.tile([66, QB], F32, tag="otp",
                                         name=f"otp_{es}_{j}_{s2}")
                               for s2 in range(2)]
                        pbuf = [None] * m

                        def emit_ot(idx, c, ot2=ot2, pbuf=pbuf, m=m, j=j, es=es):
                            for sub in range(2):
                                nc.tensor.matmul(
                                    ot2[sub][:, :],
                                    vw_c[c][:, 2 * es + sub, :],
                                    pbuf[idx][:, ts(sub, QB)],
                                    start=(idx == 0), stop=(idx == m - 1))

                        for idx, c in enumerate(chunks):
                            st2 = psST.tile([128, 2 * QB], F32, tag="st",
                                            name=f"st_{es}_{j}_{c}")
                            for sub in range(2):
                                r0 = 64 * sub
                                nc.tensor.matmul(
                                    st2[:, ts(sub, QB)],
                                    kw2[es][c // 4][r0:r0 + 64, ts(c % 4, 128)],
                                    qw2[es][j][r0:r0 + 64, :],
                                    start=True, stop=True)
                            dd = c - 4 * j
                            diag = 0 <= dd < 4
                            if diag and not post_mask:
                                for sub in range(2):
                                    nc.vector.tensor_add(
                                        st2[:, ts(sub, QB)],
                                        st2[:, ts(sub, QB)], trim[:, dd, :])
                            p = ppool.tile([128, 2 * QB], DT_ATTN, tag="p")
                            nc.scalar.activation(
                                out=p[:, :], in_=st2[:, :], func=AF.Exp,
                                bias=vmbt[:, c:c + 1], scale=0.125)
                            if diag and post_mask:
                                off = 128 * dd
                                for sub in range(2):
                                    nc.vector.tensor_mul(
                                        p[:, sub * QB + off:(sub + 1) * QB],
                                        p[:, sub * QB + off:(sub + 1) * QB],
                                        trim[:, dd, off:QB])
                            pbuf[idx] = p
                            if idx >= LAG:
                                emit_ot(idx - LAG, chunks[idx - LAG])
                            yield
                        for idx in range(max(0, m - LAG), m):
                            emit_ot(idx, chunks[idx])

                        for sub in range(2):
                            h = 2 * es + sub
                            osb = osbp.tile([66, QB], TRT, tag="osb")
                            nc.vector.tensor_copy(out=osb[:, :],
                                                  in_=ot2[sub][:, :])
                            for t in range(4):
                                qb = 4 * j + t
                                tr = psTR.tile([128, 66], TRT, tag="tr")
                                nc.tensor.transpose(
                                    tr[:, :], osb[:, ts(t, 128)],
                                    idt[0:66, 0:66])
                                rc = small.tile([128, 1], F32, tag="rc")
                                if qb in degen_qbs:
                                    dn = small.tile([128, 1], F32, tag="dn")
                                    nc.vector.tensor_add(
                                        dn[:, :], tr[:, 64:65],
                                        isdtt[:, qb:qb + 1])
                                    nc.vector.reciprocal(rc[:, :], dn[:, :])
                                else:
                                    nc.vector.reciprocal(rc[:, :], tr[:, 64:65])
                                nc.vector.tensor_scalar(
                                    out=oall[qb][:, ts(h, 64)],
                                    in0=tr[:, 0:64], scalar1=rc[:, 0:1],
                                    scalar2=qmtt[:, qb:qb + 1],
                                    op0=ALU.mult, op1=ALU.mult)
                                for i in degen_qbs.get(qb, ()):
                                    fb = fb_holder["fb"]
                                    nc.vector.scalar_tensor_tensor(
                                        out=oall[qb][:, ts(h, 64)],
                                        in0=fb[h][i][:, :],
                                        scalar=isdqt[:, qb, i:i + 1],
                                        in1=oall[qb][:, ts(h, 64)],
                                        op0=ALU.mult, op1=ALU.add)
                                if h == HG - 1:
                                    nc.sync.dma_start(
                                        out=o_d[ts(qb, 128), :],
                                        in_=oall[qb][:, :])
                        yield

            # drive: interleave v-proj chunks with the attention stream 1:1
            stream = attention_stream()
            for lt in range(C):
                emit_vproj(lt)
                next(stream, None)
            fb_holder["fb"] = emit_fix()
            for _ in stream:
                pass
            _attn_scope.close()
    nc.finalize()
    return nc


def _host_prep(q, k, v, v_mask, q_mask, Wq, Wk, Wv, precision):
    """Per-core input maps + degenerate-row bookkeeping."""
    import ml_dtypes
    dt_proj = np.float32 if precision in ("f32r", "hybrid") else ml_dtypes.bfloat16
    dt_attn = np.float32 if precision == "f32r" else ml_dtypes.bfloat16
    f32 = np.float32
    q, k, v = (np.asarray(x, f32) for x in (q, k, v))
    v_mask, q_mask = np.asarray(v_mask, f32), np.asarray(q_mask, f32)
    Wq, Wk, Wv = (np.asarray(x, f32) for x in (Wq, Wk, Wv))

    idn = np.eye(128, dtype=f32)
    oz4 = np.zeros((128, HG, 2), f32)
    oz4[:, :, 0] = 1.0
    # trimask[p, dd, x] = -8e10 where x >= p + 128*dd  (k_global <= q_global)
    p_i = np.arange(128)[:, None, None]
    dd_i = np.arange(4)[None, :, None]
    x_i = np.arange(QB)[None, None, :]
    trm = np.where(x_i >= p_i + 128 * dd_i, f32(-8.0 * BIG), f32(0.0)).astype(f32)
    tr01 = np.where(x_i >= p_i + 128 * dd_i, f32(0.0), f32(1.0)).astype(f32)

    # degenerate rows per batch + fix weight vectors
    deg, wfix_cols, isd, isdq_cols = [], [], [], []
    for b in range(B):
        vm = v_mask[b]
        rows = [qq for qq in range(L)
                if qq == L - 1 or not vm[qq + 1:].any()]
        deg.append(rows)
        cols = []
        for qq in rows:
            single = np.zeros(L, f32)
            kk = np.arange(L)
            causal = kk <= qq
            pen = causal.astype(np.int64) + (vm == 0).astype(np.int64)
            m = pen == pen.min()   # max-attaining set under -BIG penalties
            single[m] = 1.0 / m.sum()
            cols.append(single)
        wfix_cols.append(cols)

    nd = max(len(r) for r in deg)
    degen_qbs = {}
    for b in range(B):
        for i, qq in enumerate(deg[b]):
            degen_qbs.setdefault(qq // 128, set()).add(i)
    degen_qbs = {qb: sorted(s) for qb, s in degen_qbs.items()}

    WqT, WkT, WvT = Wq.T.copy(), Wk.T.copy(), Wv.T.copy()
    in_maps = []
    for core in range(NCORES):
        b, g = divmod(core, HG)
        sl = slice(E * g, E * g + E)
        vm, qm = v_mask[b], q_mask[b]
        vmb = (-BIG * (1.0 - vm)).reshape(C, 128).T.astype(f32)
        qmt = qm.reshape(C, 128).T.astype(f32)
        isd_v = np.zeros(L, f32)
        isdq_v = np.zeros((L, nd), f32)
        wfx_v = np.zeros((L, 2 * nd), f32)
        for i, qq in enumerate(deg[b]):
            isd_v[qq] = 1.0
            isdq_v[qq, i] = qm[qq]
            wfx_v[:, 2 * i] = wfix_cols[b][i]
        in_maps.append({
            "xq": np.ascontiguousarray(q[b].T.astype(dt_proj)),
            "xk": np.ascontiguousarray(k[b].T.astype(dt_proj)),
            "xv": np.ascontiguousarray(v[b].T.astype(dt_proj)),
            "wq": np.ascontiguousarray(WqT[:, sl].astype(dt_proj)),
            "wk": np.ascontiguousarray(WkT[:, sl].astype(dt_proj)),
            "wv": np.ascontiguousarray(WvT[:, sl].astype(dt_proj)),
            "idn": idn.astype(dt_attn if precision != "f32r" else np.float32), "oz4": oz4.astype(dt_attn), "trm": trm,
            "tr01": tr01.astype(dt_attn),
            "vmb": vmb, "qmt": qmt,
            "isdt": isd_v.reshape(C, 128).T.copy(),
            "isdq": np.ascontiguousarray(
                isdq_v.reshape(C, 128, nd).transpose(1, 0, 2)),
            "wfx": np.ascontiguousarray(
                wfx_v.reshape(C, 128, 2 * nd).transpose(1, 0, 2).astype(dt_attn)),
        })
    return in_maps, nd, degen_qbs


def kernel(q, k, v, v_mask, q_mask, Wq, Wk, Wv):
    global LAST_EXEC_NS, LAST_TRACE
    from concourse.bass_utils import run_bass_kernel_spmd

    in_maps, nd, degen_qbs = _host_prep(q, k, v, v_mask, q_mask, Wq, Wk, Wv,
                                        PRECISION)
    key = (PRECISION, nd,
           tuple(sorted((qb, tuple(i)) for qb, i in degen_qbs.items())))
    if key not in _CACHE:
        _CACHE[key] = _build_program(nd, degen_qbs, PRECISION)
    nc = _CACHE[key]

    kwargs = {}
    if PROFILE:
        import sys, types
        sys.path.insert(0, "/root/.axon_site/trn_agent_boot")
        import trn_boot
        raw = trn_boot._ntff_profile_via_ctypes("/opt/axon/libaxon_pjrt.so")
        mod = types.ModuleType("antenv.axon_hooks")
        mod.get_axon_ntff_profile_hook = (
            lambda: (lambda out_dir, ids: raw(out_dir, None)))
        sys.modules["antenv.axon_hooks"] = mod
        kwargs = dict(trace=True)

    res = run_bass_kernel_spmd(nc, in_maps, core_ids=list(range(NCORES)), **kwargs)
    if PROFILE:
        LAST_EXEC_NS = res.exec_time_ns
        LAST_TRACE = (res.instructions_and_trace[1]
                      if res.instructions_and_trace else None)

    out = np.empty((B, L, H * DK), np.float32)
    for core in range(NCORES):
        b, g = divmod(core, HG)
        out[b, :, E * g:E * g + E] = res.results[core]["o"]
    return out


# revision 21
# speedup vs baseline: 1.0692x; 1.0030x over previous
"""Trainium2 Bass kernel for nn_Attention_68676527063657.

Full (unsharded) multi-head attention with a quirky causal mask:
  qw = q @ Wq.T; kw = k @ Wk.T; vw = v @ Wv.T   (per-head split, dk=dv=64)
  a  = (qw . kw)/8 - (1-v_mask)*1e10 - tril(ones)*1e10   (diag included!)
  o  = softmax(a) @ vw, then o *= q_mask

Sharding: core c in [0,8): batch b = c//4, head-group g = c%4 (heads 4g..4g+4).
Each core computes o[b, :, 256g:256g+256] independently; host gathers.

Device algorithm (per core), all matmuls in float32r (full-rate fp32-ish):
  - host pre-transposes q/k/v and weights so every contraction dim lands on
    SBUF partitions; no on-chip input transposes needed.
  - scores are computed TRANSPOSED: ST[k-chunk 128, q-block 512] so that
    softmax normalization needs no P transpose before the P@V matmul.
  - v_mask folds into the exp() activation bias (per-partition = per-k).
  - causal mask: k-chunks fully below the diagonal are skipped outright;
    diagonal chunks get a precomputed -8e10 triangular add in PSUM.
  - row-sums of P come free as an extra all-ones column of V (col 64).
  - rows with no visible key (always q=2047; plus any v_mask-degenerate
    rows) are fixed up with a per-row weighted average of vw computed
    on-device from host-provided weights.
"""

import numpy as np

B, L, D = 2, 2048, 1024
H, DK = 16, 64
HG = 4            # heads per core
E = HG * DK       # 256 per-core output features
NCORES = 8
J, QB = 4, 512    # q blocks
C, KB = 16, 128   # k chunks
BIG = 1e10

_CACHE = {}
PROFILE = False
LAST_EXEC_NS = None
LAST_TRACE = None
# "bf16" (fastest), "f32r" (most accurate), "hybrid" (f32r proj, bf16 attention)
PRECISION = "bf16"


def _build_program(nd, degen_qbs, precision):
    import concourse.bass as bass
    import concourse.mybir as mybir
    from concourse import bacc
    from concourse.tile import TileContext

    F32, F32R = mybir.dt.float32, mybir.dt.float32r
    BF16 = mybir.dt.bfloat16
    DT_PROJ = F32R if precision in ("f32r", "hybrid") else BF16
    DT_ATTN = F32R if precision == "f32r" else BF16
    TRT = BF16 if DT_ATTN == BF16 else F32
    AF = mybir.ActivationFunctionType
    ALU = mybir.AluOpType
    ts = bass.ts

    nc = bacc.Bacc(None)
    xq = nc.dram_tensor("xq", [D, L], DT_PROJ, kind="ExternalInput")
    xk = nc.dram_tensor("xk", [D, L], DT_PROJ, kind="ExternalInput")
    xv = nc.dram_tensor("xv", [D, L], DT_PROJ, kind="ExternalInput")
    wq = nc.dram_tensor("wq", [D, E], DT_PROJ, kind="ExternalInput")
    wk = nc.dram_tensor("wk", [D, E], DT_PROJ, kind="ExternalInput")
    wv = nc.dram_tensor("wv", [D, E], DT_PROJ, kind="ExternalInput")
    idn = nc.dram_tensor("idn", [128, 128], TRT, kind="ExternalInput")
    oz4 = nc.dram_tensor("oz4", [128, HG, 2], DT_ATTN, kind="ExternalInput")
    trm = nc.dram_tensor("trm", [128, 4, QB], F32, kind="ExternalInput")
    tr01 = nc.dram_tensor("tr01", [128, 4, QB], DT_ATTN, kind="ExternalInput")
    vmb = nc.dram_tensor("vmb", [128, C], F32, kind="ExternalInput")
    qmt = nc.dram_tensor("qmt", [128, C], F32, kind="ExternalInput")
    isdt = nc.dram_tensor("isdt", [128, C], F32, kind="ExternalInput")
    isdq = nc.dram_tensor("isdq", [128, C, nd], F32, kind="ExternalInput")
    wfx = nc.dram_tensor("wfx", [128, C, 2 * nd], DT_ATTN, kind="ExternalInput")
    o_d = nc.dram_tensor("o", [L, E], F32, kind="ExternalOutput")

    with TileContext(nc) as tc:
        with tc.tile_pool(name="consts", bufs=1) as consts, \
             tc.tile_pool(name="xt", bufs=17) as xtp, \
             tc.tile_pool(name="qk2", bufs=1) as qk2p, \
             tc.tile_pool(name="pp", bufs=8) as ppool, \
             tc.tile_pool(name="osb", bufs=3) as osbp, \
             tc.tile_pool(name="oall", bufs=17) as oallp, \
             tc.tile_pool(name="small", bufs=4) as small, \
             tc.tile_pool(name="fbp", bufs=1) as fbp:

            post_mask = (DT_ATTN != F32R)
            wdram = {"q": wq, "k": wk, "v": wv}
            wsb = {}

            # per-block activation tiles (fine-grained deps for interleaving)
            qw2 = [[qk2p.tile([128, QB], DT_ATTN, tag=f"qw2_{es}_{lc}",
                              name=f"qw2_{es}_{lc}") for lc in range(4)]
                   for es in range(2)]
            kw2 = [[qk2p.tile([128, QB], DT_ATTN, tag=f"kw2_{es}_{lc}",
                              name=f"kw2_{es}_{lc}") for lc in range(4)]
                   for es in range(2)]
            vw_c = [qk2p.tile([128, HG, 66], DT_ATTN, tag=f"vw_{c}",
                              name=f"vw_{c}") for c in range(C)]

            # ---------------- q/k projections (PE-dense prologue) ------------
            from contextlib import ExitStack
            _psa_scope = ExitStack()
            psA = _psa_scope.enter_context(
                tc.tile_pool(name="psA", bufs=8, space="PSUM"))
            xtv = None
            for nm in ("q", "k"):
                xin = {"q": xq, "k": xk}[nm]
                wsb[nm] = consts.tile([128, 8, E], DT_PROJ, tag=f"w_{nm}",
                                      name=f"wsb_{nm}")
                nc.sync.dma_start(
                    out=wsb[nm][:, :, :],
                    in_=wdram[nm].rearrange("(t p) e -> p t e", p=128))
                xt = [xtp.tile([128, L], DT_PROJ, tag="xt", name=f"xt_{nm}_{d2}")
                      for d2 in range(8)]
                for d in range(8):
                    nc.sync.dma_start(out=xt[d][:, :], in_=xin[ts(d, 128), :])
                if nm == "k":
                    wsb["v"] = consts.tile([128, 8, E], DT_PROJ, tag="w_v",
                                           name="wsb_v")
                    nc.sync.dma_start(
                        out=wsb["v"][:, :, :],
                        in_=wdram["v"].rearrange("(t p) e -> p t e", p=128))
                    xtv = [xtp.tile([128, L], DT_PROJ, tag="xt",
                                    name=f"xt_v_{d2}") for d2 in range(8)]
                    for d in range(8):
                        nc.sync.dma_start(out=xtv[d][:, :], in_=xv[ts(d, 128), :])
                dst = qw2 if nm == "q" else kw2
                pss = [[psA.tile([128, QB], F32, tag="ppqk",
                                 name=f"ppqk_{nm}_{es}_{lc}")
                        for lc in range(4)] for es in range(2)]
                for d in range(8):
                    for es in range(2):
                        for lc in range(4):
                            nc.tensor.matmul(
                                pss[es][lc][:, :],
                                wsb[nm][:, d, ts(es, 128)],
                                xt[d][:, ts(lc, QB)],
                                start=(d == 0), stop=(d == 7))
                for es in range(2):
                    for lc in range(4):
                        nc.scalar.copy(out=dst[es][lc][:, :],
                                       in_=pss[es][lc][:, :])

            _psa_scope.close()
            _attn_scope = ExitStack()
            psST = _attn_scope.enter_context(
                tc.tile_pool(name="psST", bufs=2, space="PSUM"))
            psV = _attn_scope.enter_context(
                tc.tile_pool(name="psV", bufs=1, space="PSUM"))
            psOT = _attn_scope.enter_context(
                tc.tile_pool(name="psOT", bufs=2, space="PSUM"))
            psTR = _attn_scope.enter_context(
                tc.tile_pool(name="psTR", bufs=1, space="PSUM"))

            # late consts (needed from attention on)
            idt = consts.tile([128, 128], TRT, tag="idt")
            nc.sync.dma_start(out=idt[:, :], in_=idn[:, :])
            if post_mask:
                trim = consts.tile([128, 4, QB], DT_ATTN, tag="trim")
                nc.sync.dma_start(out=trim[:, :, :], in_=tr01[:, :, :])
            else:
                trim = consts.tile([128, 4, QB], F32, tag="trim")
                nc.sync.dma_start(out=trim[:, :, :], in_=trm[:, :, :])
            vmbt = consts.tile([128, C], F32, tag="vmbt")
            nc.sync.dma_start(out=vmbt[:, :], in_=vmb[:, :])
            qmtt = consts.tile([128, C], F32, tag="qmtt")
            nc.sync.dma_start(out=qmtt[:, :], in_=qmt[:, :])
            isdtt = consts.tile([128, C], F32, tag="isdtt")
            nc.sync.dma_start(out=isdtt[:, :], in_=isdt[:, :])
            isdqt = consts.tile([128, C, nd], F32, tag="isdqt")
            nc.sync.dma_start(out=isdqt[:, :, :], in_=isdq[:, :, :])
            wfxt = consts.tile([128, C, 2 * nd], DT_ATTN, tag="wfxt")
            nc.sync.dma_start(out=wfxt[:, :, :], in_=wfx[:, :, :])

            def emit_vproj(lt):
                ps = psV.tile([128, E], F32, tag="ppv", name=f"ppv_{lt}")
                for d in range(8):
                    nc.tensor.matmul(
                        ps[:, :], xtv[d][:, ts(lt, 128)], wsb["v"][:, d, :],
                        start=(d == 0), stop=(d == 7))
                nc.vector.tensor_copy(
                    out=vw_c[lt][:, :, 0:64],
                    in_=ps[:, :].rearrange("p (h e) -> p h e", h=HG))
                nc.sync.dma_start(out=vw_c[lt][:, :, 64:66], in_=oz4[:, :, :])

            def emit_fix():
                fb = [[None] * nd for _ in range(HG)]
                for i in range(nd):
                    for h in range(HG):
                        pf = psOT.tile([2, 64], F32, tag="otp",
                                       name=f"pf_{i}_{h}")
                        for c in range(C):
                            nc.tensor.matmul(
                                pf[:, :],
                                wfxt[:, c, 2 * i:2 * i + 2],
                                vw_c[c][:, h, 0:64],
                                start=(c == 0), stop=(c == C - 1))
                        fr = small.tile([1, 64], F32, tag="fixrow")
                        nc.vector.tensor_copy(out=fr[:, :], in_=pf[0:1, :])
                        t = fbp.tile([128, 64], F32, tag=f"fb_{h}_{i}")
                        nc.gpsimd.partition_broadcast(t[:, :], fr[0:1, :])
                        fb[h][i] = t
                return fb

            # attention stream generator: yields after each chunk emission
            LAG = 5
            oall = [oallp.tile([128, E], F32, tag="oall", name=f"oall_{qb}")
                    for qb in range(C)]
            fb_holder = {}

            def attention_stream():
                for es in range(2):
                    for j in (range(J) if es == 0 else reversed(range(J))):
                        chunks = list(range(4 * j, C))
                        m = len(chunks)
                        ot2 = [psOT.tile([66, QB], F32, tag="otp",
                                         name=f"otp_{es}_{j}_{s2}")
                               for s2 in range(2)]
                        pbuf = [None] * m

                        def emit_ot(idx, c, ot2=ot2, pbuf=pbuf, m=m, j=j, es=es):
                            for sub in range(2):
                                nc.tensor.matmul(
                                    ot2[sub][:, :],
                                    vw_c[c][:, 2 * es + sub, :],
                                    pbuf[idx][:, ts(sub, QB)],
                                    start=(idx == 0), stop=(idx == m - 1))

                        for idx, c in enumerate(chunks):
                            st2 = psST.tile([128, 2 * QB], F32, tag="st",
                                            name=f"st_{es}_{j}_{c}")
                            for sub in range(2):
                                r0 = 64 * sub
                                nc.tensor.matmul(
                                    st2[:, ts(sub, QB)],
                                    kw2[es][c // 4][r0:r0 + 64, ts(c % 4, 128)],
                                    qw2[es][j][r0:r0 + 64, :],
                                    start=True, stop=True)
                            dd = c - 4 * j
                            diag = 0 <= dd < 4
                            if diag and not post_mask:
                                for sub in range(2):
                                    nc.vector.tensor_add(
                                        st2[:, ts(sub, QB)],
                                        st2[:, ts(sub, QB)], trim[:, dd, :])
                            p = ppool.tile([128, 2 * QB], DT_ATTN, tag="p")
                            nc.scalar.activation(
                                out=p[:, :], in_=st2[:, :], func=AF.Exp,
                                bias=vmbt[:, c:c + 1], scale=0.125)
                            if diag and post_mask:
                                off = 128 * dd
                                for sub in range(2):
                                    nc.vector.tensor_mul(
                                        p[:, sub * QB + off:(sub + 1) * QB],
                                        p[:, sub * QB + off:(sub + 1) * QB],
                                        trim[:, dd, off:QB])
                            pbuf[idx] = p
                            if idx >= LAG:
                                emit_ot(idx - LAG, chunks[idx - LAG])
                            yield
                        for idx in range(max(0, m - LAG), m):
                            emit_ot(idx, chunks[idx])

                        last_group = (es == 1 and j == 0)
                        for sub in range(2):
                            h = 2 * es + sub
                            osb = osbp.tile([66, QB], TRT, tag="osb")
                            nc.vector.tensor_copy(out=osb[:, :],
                                                  in_=ot2[sub][:, :])
                            for t in range(4):
                                qb = 4 * j + t
                                if last_group:
                                    tr = psOT.tile([128, 66], TRT, tag="otp",
                                                   name=f"trL_{sub}_{t}")
                                else:
                                    tr = psTR.tile([128, 66], TRT, tag="tr")
                                nc.tensor.transpose(
                                    tr[:, :], osb[:, ts(t, 128)],
                                    idt[0:66, 0:66])
                                rc = small.tile([128, 1], F32, tag="rc")
                                if qb in degen_qbs:
                                    dn = small.tile([128, 1], F32, tag="dn")
                                    nc.vector.tensor_add(
                                        dn[:, :], tr[:, 64:65],
                                        isdtt[:, qb:qb + 1])
                                    nc.vector.reciprocal(rc[:, :], dn[:, :])
                                else:
                                    nc.vector.reciprocal(rc[:, :], tr[:, 64:65])
                                nc.vector.tensor_scalar(
                                    out=oall[qb][:, ts(h, 64)],
                                    in0=tr[:, 0:64], scalar1=rc[:, 0:1],
                                    scalar2=qmtt[:, qb:qb + 1],
                                    op0=ALU.mult, op1=ALU.mult)
                                for i in degen_qbs.get(qb, ()):
                                    fb = fb_holder["fb"]
                                    nc.vector.scalar_tensor_tensor(
                                        out=oall[qb][:, ts(h, 64)],
                                        in0=fb[h][i][:, :],
                                        scalar=isdqt[:, qb, i:i + 1],
                                        in1=oall[qb][:, ts(h, 64)],
                                        op0=ALU.mult, op1=ALU.add)
                                if h == HG - 1:
                                    nc.sync.dma_start(
                                        out=o_d[ts(qb, 128), :],
                                        in_=oall[qb][:, :])
                        yield

            # drive: interleave v-proj chunks with the attention stream 1:1
            stream = attention_stream()
            for lt in range(C):
                emit_vproj(lt)
                next(stream, None)
            fb_holder["fb"] = emit_fix()
            for _ in stream:
                pass
            _attn_scope.close()
    nc.finalize()
    return nc


def _host_prep(q, k, v, v_mask, q_mask, Wq, Wk, Wv, precision):
    """Per-core input maps + degenerate-row bookkeeping."""
    import ml_dtypes
    dt_proj = np.float32 if precision in ("f32r", "hybrid") else ml_dtypes.bfloat16
    dt_attn = np.float32 if precision == "f32r" else ml_dtypes.bfloat16
    f32 = np.float32
    q, k, v = (np.asarray(x, f32) for x in (q, k, v))
    v_mask, q_mask = np.asarray(v_mask, f32), np.asarray(q_mask, f32)
    Wq, Wk, Wv = (np.asarray(x, f32) for x in (Wq, Wk, Wv))

    idn = np.eye(128, dtype=f32)
    oz4 = np.zeros((128, HG, 2), f32)
    oz4[:, :, 0] = 1.0
    # trimask[p, dd, x] = -8e10 where x >= p + 128*dd  (k_global <= q_global)
    p_i = np.arange(128)[:, None, None]
    dd_i = np.arange(4)[None, :, None]
    x_i = np.arange(QB)[None, None, :]
    trm = np.where(x_i >= p_i + 128 * dd_i, f32(-8.0 * BIG), f32(0.0)).astype(f32)
    tr01 = np.where(x_i >= p_i + 128 * dd_i, f32(0.0), f32(1.0)).astype(f32)

    # degenerate rows per batch + fix weight vectors
    deg, wfix_cols, isd, isdq_cols = [], [], [], []
    for b in range(B):
        vm = v_mask[b]
        rows = [qq for qq in range(L)
                if qq == L - 1 or not vm[qq + 1:].any()]
        deg.append(rows)
        cols = []
        for qq in rows:
            single = np.zeros(L, f32)
            kk = np.arange(L)
            causal = kk <= qq
            pen = causal.astype(np.int64) + (vm == 0).astype(np.int64)
            m = pen == pen.min()   # max-attaining set under -BIG penalties
            single[m] = 1.0 / m.sum()
            cols.append(single)
        wfix_cols.append(cols)

    nd = max(len(r) for r in deg)
    degen_qbs = {}
    for b in range(B):
        for i, qq in enumerate(deg[b]):
            degen_qbs.setdefault(qq // 128, set()).add(i)
    degen_qbs = {qb: sorted(s) for qb, s in degen_qbs.items()}

    WqT, WkT, WvT = Wq.T.copy(), Wk.T.copy(), Wv.T.copy()
    in_maps = []
    for core in range(NCORES):
        b, g = divmod(core, HG)
        sl = slice(E * g, E * g + E)
        vm, qm = v_mask[b], q_mask[b]
        vmb = (-BIG * (1.0 - vm)).reshape(C, 128).T.astype(f32)
        qmt = qm.reshape(C, 128).T.astype(f32)
        isd_v = np.zeros(L, f32)
        isdq_v = np.zeros((L, nd), f32)
        wfx_v = np.zeros((L, 2 * nd), f32)
        for i, qq in enumerate(deg[b]):
            isd_v[qq] = 1.0
            isdq_v[qq, i] = qm[qq]
            wfx_v[:, 2 * i] = wfix_cols[b][i]
        in_maps.append({
            "xq": np.ascontiguousarray(q[b].T.astype(dt_proj)),
            "xk": np.ascontiguousarray(k[b].T.astype(dt_proj)),
            "xv": np.ascontiguousarray(v[b].T.astype(dt_proj)),
            "wq": np.ascontiguousarray(WqT[:, sl].astype(dt_proj)),
            "wk": np.ascontiguousarray(WkT[:, sl].astype(dt_proj)),
            "wv": np.ascontiguousarray(WvT[:, sl].astype(dt_proj)),
            "idn": idn.astype(dt_attn if precision != "f32r" else np.float32), "oz4": oz4.astype(dt_attn), "trm": trm,
            "tr01": tr01.astype(dt_attn),
            "vmb": vmb, "qmt": qmt,
            "isdt": isd_v.reshape(C, 128).T.copy(),
            "isdq": np.ascontiguousarray(
                isdq_v.reshape(C, 128, nd).transpose(1, 0, 2)),
            "wfx": np.ascontiguousarray(
                wfx_v.reshape(C, 128, 2 * nd).transpose(1, 0, 2).astype(dt_attn)),
        })
    return in_maps, nd, degen_qbs


def kernel(q, k, v, v_mask, q_mask, Wq, Wk, Wv):
    global LAST_EXEC_NS, LAST_TRACE
    from concourse.bass_utils import run_bass_kernel_spmd

    in_maps, nd, degen_qbs = _host_prep(q, k, v, v_mask, q_mask, Wq, Wk, Wv,
                                        PRECISION)
    key = (PRECISION, nd,
           tuple(sorted((qb, tuple(i)) for qb, i in degen_qbs.items())))
    if key not in _CACHE:
        _CACHE[key] = _build_program(nd, degen_qbs, PRECISION)
    nc = _CACHE[key]

    kwargs = {}
    if PROFILE:
        import sys, types
        sys.path.insert(0, "/root/.axon_site/trn_agent_boot")
        import trn_boot
        raw = trn_boot._ntff_profile_via_ctypes("/opt/axon/libaxon_pjrt.so")
        mod = types.ModuleType("antenv.axon_hooks")
        mod.get_axon_ntff_profile_hook = (
            lambda: (lambda out_dir, ids: raw(out_dir, None)))
        sys.modules["antenv.axon_hooks"] = mod
        kwargs = dict(trace=True)

    res = run_bass_kernel_spmd(nc, in_maps, core_ids=list(range(NCORES)), **kwargs)
    if PROFILE:
        LAST_EXEC_NS = res.exec_time_ns
        LAST_TRACE = (res.instructions_and_trace[1]
                      if res.instructions_and_trace else None)

    out = np.empty((B, L, H * DK), np.float32)
    for core in range(NCORES):
        b, g = divmod(core, HG)
        out[b, :, E * g:E * g + E] = res.results[core]["o"]
    return out
